# revision 1
# baseline (speedup 1.0000x reference)
"""Contrastive (SimCLR-style) loss on 8 Trainium2 NeuronCores.

Math (matches the reference exactly):
  P = concat(projection1, projection2)            # [8192, 256]
  sim = cos_sim(P_i, P_j); diag masked to -1e9; logits = sim / 0.5
  labels = arange(2B)  -> picks the masked diagonal, so
  loss = -mean_i( logp_ii ),  logp_ii = f32(-2e9 - lse_i),
  lse_i = log(sum_{j != i} exp(2*sim_ij))

Distribution: data-parallel over the 8192 rows.  Each core receives the
full projection matrix (row-major fp32 for norms + pre-transposed bf16
for the matmul operand) plus its own 1024-row block.  On chip it:
  - computes row norms (DVE square+reduce, Newton rsqrt -- no ScalarE),
  - scales the transposed operand by 1/norm (bf16),
  - matmuls its row block against all 8192 columns (bf16, fp32 PSUM),
  - streams exp through ScalarE with fused row-sum accumulation,
  - subtracts the diagonal term and takes log.
Host all-reduces the per-row lse partials and applies the reference's
fp32 arithmetic for the final mean.
"""

import sys

for _p in ("/opt/trn_rl_repo", "/root/.axon_site/_ro/trn_rl_repo"):
    if _p not in sys.path:
        sys.path.append(_p)

import numpy as np

import concourse.bacc as bacc
import concourse.tile as tile
from concourse import mybir
from concourse import bass_utils

F32 = mybir.dt.float32
BF16 = mybir.dt.bfloat16
I32 = mybir.dt.int32
AF = mybir.ActivationFunctionType
ALU = mybir.AluOpType

N_CORES = 8
B = 8192          # total rows (2 * batch)
D = 256           # projection dim
BLK = B // N_CORES        # 1024 rows per core
M_TILES = BLK // 128      # 8 row tiles per core
N_COLS = 512              # matmul free dim (one PSUM bank)
GROUP = 2048              # ACT exp batch (4 PSUM banks) = one column group
N_GROUPS = B // GROUP     # 4
N_PER_GROUP = GROUP // N_COLS  # 4
U = 16                    # consecutive rows per partition in stats loads
RSQRT_MAGIC = 0x5F3759DF


def _newton_rsqrt(nc, pool, out_rn, s):
    """out_rn = 1/sqrt(s), entirely on VectorE (fp32).

    Quake-style bit seed + 2 Newton iterations (~5e-6 rel err).  Keeps
    ScalarE free for exp and avoids sqrt<->exp table reloads.
    """
    p, w = s.shape
    ibits = pool.tile([p, w], I32, name="ibits", tag="rsq_i", bufs=2)
    nc.vector.tensor_scalar(
        out=ibits, in0=s.bitcast(I32), scalar1=1, scalar2=None,
        op0=ALU.arith_shift_right,
    )
    nc.vector.tensor_scalar(
        out=ibits, in0=ibits, scalar1=-1, scalar2=RSQRT_MAGIC,
        op0=ALU.mult, op1=ALU.add,
    )
    y = ibits.bitcast(F32)
    t1 = pool.tile([p, w], F32, name="t1", tag="rsq_t1", bufs=2)
    for _ in range(2):
        nc.vector.tensor_mul(t1, y, y)
        nc.vector.tensor_mul(t1, t1, s)
        nc.vector.tensor_scalar(
            out=t1, in0=t1, scalar1=-0.5, scalar2=1.5,
            op0=ALU.mult, op1=ALU.add,
        )
        nc.vector.tensor_mul(y, y, t1)
    nc.vector.tensor_copy(out_rn, y)


def _emit(tc, p_stats, pt, p_blk, eye_in, lse_out):
    nc = tc.nc

    persist = tc.alloc_tile_pool(name="persist", bufs=1)
    pin = tc.alloc_tile_pool(name="pin", bufs=2)
    work = tc.alloc_tile_pool(name="work", bufs=2)
    dram = tc.alloc_tile_pool(name="dram", bufs=1, space="DRAM")
    epool = tc.alloc_tile_pool(name="epool", bufs=2)

    # Persistent tensors
    qt0 = persist.tile([128, B], BF16, tag="qt0", name="qt0")
    qt1 = persist.tile([128, B], BF16, tag="qt1", name="qt1")
    bt0 = persist.tile([128, BLK], BF16, tag="bt0", name="bt0")
    bt1 = persist.tile([128, BLK], BF16, tag="bt1", name="bt1")
    q_b = persist.tile([128, M_TILES, D], BF16, tag="q_b", name="q_b")
    rn_f = persist.tile([128, 64], F32, tag="rn_f", name="rn_f")
    rn_b = persist.tile([128, M_TILES], F32, tag="rn_b", name="rn_b")
    selfdot = persist.tile([128, M_TILES], F32, tag="selfdot", name="selfdot")
    sums = persist.tile([128, N_GROUPS * M_TILES], F32, tag="sums", name="sums")
    rowsum = persist.tile([128, M_TILES], F32, tag="rowsum", name="rowsum")
    exps = persist.tile([128, M_TILES], F32, tag="exps", name="exps")
    lse = persist.tile([128, M_TILES], F32, tag="lse", name="lse")
    dram_rn = dram.tile([B], F32, tag="dram_rn", name="dram_rn")

    # ---- This core's row block: norms, scale, self-dot, transpose ----
    pb = p_blk.rearrange("(t p) d -> t p d", p=128)    # [8, 128, 256]
    blk = persist.tile([128, M_TILES, D], F32, tag="blk", name="blk")
    eye = persist.tile([128, 128], BF16, tag="eye", name="eye")
    nc.gpsimd.dma_start(out=eye, in_=eye_in)
    for t in range(M_TILES):
        nc.gpsimd.dma_start(out=blk[:, t, :], in_=pb[t])
    sq_b = work.tile([128, M_TILES, D], BF16, name="sq_b", tag="sq_b", bufs=1)
    nc.vector.tensor_mul(sq_b, blk, blk)
    stats_b = work.tile([128, M_TILES], F32, name="stats_b", tag="stats_b", bufs=1)
    nc.vector.tensor_reduce(stats_b, sq_b, axis=mybir.AxisListType.X, op=ALU.add)
    _newton_rsqrt(nc, work, rn_b, stats_b)
    for t in range(M_TILES):
        nc.vector.tensor_scalar_mul(q_b[:, t, :], blk[:, t, :], rn_b[:, t : t + 1])
    sq_b2 = work.tile([128, M_TILES, D], BF16, name="sq_b2", tag="sq_b", bufs=1)
    nc.vector.tensor_mul(sq_b2, q_b, q_b)
    nc.vector.tensor_reduce(selfdot, sq_b2, axis=mybir.AxisListType.X, op=ALU.add)
    # Transpose the block on the (otherwise idle) tensor engine; copy the
    # PSUM results to SBUF on the scalar engine.  This keeps the slow DMA
    # xbar out of the picture and frees the main loop from DMA-queue deps.
    tp_psum = tc.alloc_tile_pool(name="tp_psum", bufs=4, space="PSUM")
    for t in range(M_TILES):
        for half, btk in ((0, bt0), (1, bt1)):
            tp = tp_psum.tile([128, 128], BF16, name="tp")
            nc.tensor.transpose(tp, q_b[:, t, half * 128 : half * 128 + 128], eye)
            nc.scalar.copy(out=btk[:, t * 128 : (t + 1) * 128], in_=tp)
    tp_psum.release()
    psum_pool = tc.alloc_tile_pool(name="psum", bufs=2, space="PSUM")

    # ---- Full-matrix norms + scaled transposed operand, one group at a
    # time (group g covers columns [2048g, 2048(g+1)) = rows with the
    # same indices; the u=16 interleave keeps j-order identity) ----
    # stats load: row j = 2048t + 16p + u  ->  tile t, partition p, slot u
    ps4 = p_stats.rearrange("(t p u) d -> t p (u d)", p=128, u=U)  # [4,128,4096]
    # rn store: dram_rn[2048t + 16p + u] <- rn_small[p, 16t + u]
    rn_store = dram_rn.rearrange("(t p u) -> t p u", p=128, u=U)   # [4,128,16]

    def normalize_group(g):
        pst = pin.tile([128, U * D], F32, name="pst", tag="pst", bufs=2)
        nc.sync.dma_start(out=pst, in_=ps4[g])
        sq = work.tile([128, U * D], BF16, name="sq", tag="sq", bufs=2)
        nc.vector.tensor_mul(sq, pst, pst)
        nc.vector.tensor_reduce(
            rn_f[:, g * U : (g + 1) * U],
            sq.rearrange("p (u d) -> p u d", u=U),
            axis=mybir.AxisListType.X,
            op=ALU.add,
        )
        _newton_rsqrt(
            nc, work, rn_f[:, g * U : (g + 1) * U], rn_f[:, g * U : (g + 1) * U]
        )
        nc.sync.dma_start(
            out=rn_store[g],
            in_=rn_f[:, g * U : (g + 1) * U].rearrange("p (t u) -> p t u", u=U),
        )
        rnb = work.tile([128, GROUP], F32, name="rnb", tag="rnb", bufs=2)
        nc.sync.dma_start(
            out=rnb,
            in_=dram_rn[g * GROUP : (g + 1) * GROUP].partition_broadcast(128),
        )
        for k, qtk in enumerate((qt0, qt1)):
            ptc = pin.tile([128, GROUP], F32, name="ptc", tag="ptc", bufs=4)
            nc.gpsimd.dma_start(
                out=ptc,
                in_=pt[k * 128 : (k + 1) * 128, g * GROUP : (g + 1) * GROUP],
            )
            nc.vector.tensor_mul(
                qtk[:, g * GROUP : (g + 1) * GROUP], ptc, rnb
            )

    normalize_group(0)

    # ---- Main loop: S-block matmuls + fused exp/row-sum ----
    for g in range(N_GROUPS):
        if g + 1 < N_GROUPS:
            normalize_group(g + 1)
        for m in range(M_TILES):
            ps = psum_pool.tile([128, GROUP], F32, name="ps")
            for n4 in range(N_PER_GROUP):
                col = g * GROUP + n4 * N_COLS
                for k, (btk, qtk) in enumerate(((bt0, qt0), (bt1, qt1))):
                    nc.tensor.matmul(
                        ps[:, n4 * N_COLS : (n4 + 1) * N_COLS],
                        btk[:, m * 128 : (m + 1) * 128],
                        qtk[:, col : col + N_COLS],
                        start=(k == 0),
                        stop=(k == 1),
                    )
            esc = epool.tile([128, GROUP], BF16, name="esc")
            nc.scalar.activation(
                out=esc,
                in_=ps,
                func=AF.Exp,
                scale=2.0,
                accum_out=sums[:, g * M_TILES + m : g * M_TILES + m + 1],
            )

    # ---- Epilogue: rowsum over groups, drop diagonal, log, write out ----
    sums3 = sums.rearrange("p (g m) -> p m g", g=N_GROUPS)
    nc.vector.tensor_reduce(rowsum, sums3, axis=mybir.AxisListType.X, op=ALU.add)
    nc.scalar.activation(out=exps, in_=selfdot, func=AF.Exp, scale=2.0)
    nc.vector.tensor_tensor(lse, rowsum, exps, op=ALU.subtract)
    nc.scalar.activation(out=lse, in_=lse, func=AF.Ln)
    nc.sync.dma_start(out=lse_out, in_=lse)

    for p in (epool, psum_pool, dram, work, pin, persist):
        p.release()


_BUILT = None


def _build():
    global _BUILT
    if _BUILT is None:
        nc = bacc.Bacc("TRN2", target_bir_lowering=False, debug=False,
                       num_devices=N_CORES)
        p_stats = nc.dram_tensor("p_stats", [B, D], F32, kind="ExternalInput").ap()
        pt = nc.dram_tensor("pt", [D, B], F32, kind="ExternalInput").ap()
        eye = nc.dram_tensor("eye", [128, 128], BF16, kind="ExternalInput").ap()
        p_blk = nc.dram_tensor("p_blk", [BLK, D], F32, kind="ExternalInput").ap()
        lse_out = nc.dram_tensor("lse_out", [128, M_TILES], F32,
                                 kind="ExternalOutput").ap()
        with tile.TileContext(nc) as tc:
            _emit(tc, p_stats, pt, p_blk, eye, lse_out)
        nc.finalize()
        _BUILT = nc
    return _BUILT


def run_on_hw(P, **spmd_kwargs):
    import jax.numpy as jnp

    nc = _build()
    pt_f32 = np.ascontiguousarray(P.T)
    eye = np.asarray(jnp.eye(128, dtype=jnp.bfloat16))
    in_maps = [
        {
            "p_stats": P,
            "pt": pt_f32,
            "p_blk": np.ascontiguousarray(P[c * BLK : (c + 1) * BLK]),
            "eye": eye,
        }
        for c in range(N_CORES)
    ]
    return bass_utils.run_bass_kernel_spmd(
        nc, in_maps, core_ids=list(range(N_CORES)), **spmd_kwargs
    )


def kernel(embedding1, embedding2, projection1, projection2):
    import jax.numpy as jnp

    # embeddings are unused by the reference computation
    P = np.ascontiguousarray(
        np.concatenate([projection1, projection2], axis=0), dtype=np.float32
    )
    res = run_on_hw(P)
    # reassemble per-row lse: core c, tile column m, partition p ->
    # global row c*1024 + m*128 + p
    lse_rows = np.empty(B, np.float32)
    for c in range(N_CORES):
        arr = np.asarray(res.results[c]["lse_out"])  # [128, M_TILES]
        lse_rows[c * BLK : (c + 1) * BLK] = arr.T.reshape(-1)
    # Reference fp32 semantics: logp_ii = f32(-2e9 - lse_i) (== -2e9 for
    # any |lse| < 128), then loss = -mean(logp) with the platform's XLA
    # fp32 reduction -- reproduce it bit-for-bit.
    logp = (np.float32(-2.0e9) - lse_rows).astype(np.float32)
    loss = -jnp.mean(jnp.asarray(logp))
    return np.asarray(loss)



# revision 15
# speedup vs baseline: 1.5002x; 1.5002x over previous
"""Contrastive (SimCLR-style) loss on 8 Trainium2 NeuronCores.

Math (matches the reference exactly):
  P = concat(projection1, projection2)            # [8192, 256]
  sim = cos_sim(P_i, P_j); diag masked to -1e9; logits = sim / 0.5
  labels = arange(2B)  -> picks the masked diagonal, so
  loss = -mean_i( logp_ii ),  logp_ii = f32(-2e9 - lse_i),
  lse_i = log(sum_{j != i} exp(2*sim_ij))

Distribution: symmetric circulant scheme over 16 row blocks of 512.
exp(2*sim) is symmetric, so each unordered pair {i,j} is computed ONCE
and credited to both row i's and row j's softmax sum.  Core c owns row
blocks c and c+8; with its column space rotated left by 512c it
computes (in local columns):
  rows A = cols [0,512)     x  cols [0,4608)     (distances 0..8)
  rows B = cols [4096,4608) x  cols [4096,8192)  (distances 0..7)
Row partials come from fused ACT accumulation; the transpose credit
comes from column sums of the exp tiles (ones-matmul), excluding each
side's own diagonal block.  Host adds row+col partials (8+15 vectors
per core), subtracts the self-similarity term e^2, takes log.

On-chip per core:
  - norms of all 8192 columns from a row-major bf16 aux input via DVE
    scalar_tensor_tensor (x*x with fused accum, 4x mode), Newton rsqrt,
  - normalized operand Q in fp8e4, DoubleRow layout [128, 2, 8192]
    (d = 128t + p), built by DVE/GpSimd column-scaling,
  - fp8 DoubleRow matmuls: full K=256 contraction per instruction at
    0.5 cycles/col (157 TF/s),
  - ScalarE exp (scale=2.0) PSUM->SBUF(fp8) with accum_out row sums,
  - column sums: DoubleRow ones-matmul over fp8 exp pairs.
"""

import sys

for _p in ("/opt/trn_rl_repo", "/root/.axon_site/_ro/trn_rl_repo"):
    if _p not in sys.path:
        sys.path.append(_p)

import numpy as np

import concourse.bacc as bacc
import concourse.tile as tile
from concourse import mybir
from concourse import bass_utils

F32 = mybir.dt.float32
BF16 = mybir.dt.bfloat16
FP8 = mybir.dt.float8e4
I32 = mybir.dt.int32
AF = mybir.ActivationFunctionType
ALU = mybir.AluOpType
DR = mybir.MatmulPerfMode.DoubleRow

N_CORES = 8
B = 8192          # total rows (2 * batch)
D = 256           # projection dim
BLK = 512         # circulant row-block unit
G = 2048          # prologue column group
NG = B // G       # 4
AW = 4608         # A-side rhs window width (9 blocks, distances 0..8)
BW = 4096         # B-side rhs window width (8 blocks, distances 0..7)
CS_A = AW - BLK   # 4096 column-sum cols on the A side
CS_B = BW - BLK   # 3584 column-sum cols on the B side
CHUNK = 512       # matmul free-dim chunk (one PSUM bank)
PTILE = 1536      # PSUM tile (3 banks) = one exp instruction
RSQRT_MAGIC = 0x5F3759DF


def _newton_rsqrt(nc, pool, out_rn, s, iters=2):
    """out_rn = 1/sqrt(s), entirely on VectorE (fp32)."""
    p, w = s.shape
    ibits = pool.tile([p, w], I32, name="ibits", tag="rsq_i", bufs=2)
    nc.vector.tensor_scalar(
        out=ibits, in0=s.bitcast(I32), scalar1=1, scalar2=None,
        op0=ALU.arith_shift_right,
    )
    nc.vector.tensor_scalar(
        out=ibits, in0=ibits, scalar1=-1, scalar2=RSQRT_MAGIC,
        op0=ALU.mult, op1=ALU.add,
    )
    y = ibits.bitcast(F32)
    t1 = pool.tile([p, w], F32, name="t1", tag="rsq_t1", bufs=2)
    for _ in range(iters):
        nc.vector.tensor_mul(t1, y, y)
        nc.vector.tensor_mul(t1, t1, s)
        nc.vector.tensor_scalar(
            out=t1, in0=t1, scalar1=-0.5, scalar2=1.5,
            op0=ALU.mult, op1=ALU.add,
        )
        nc.vector.tensor_mul(y, y, t1)
    nc.vector.tensor_copy(out_rn, y)


def _emit(tc, pt_in, prow_in, rs_out, cs_out):
    nc = tc.nc

    persist = tc.alloc_tile_pool(name="persist", bufs=1)
    work = tc.alloc_tile_pool(name="work", bufs=2)
    dram = tc.alloc_tile_pool(name="dram", bufs=1, space="DRAM")
    main_psum = tc.alloc_tile_pool(name="mpsum", bufs=2, space="PSUM")
    cs_psum = tc.alloc_tile_pool(name="cpsum", bufs=2, space="PSUM")

    pt_g = [persist.tile([128, 2, G], BF16, name=f"pt{g}", tag=f"pt{g}")
            for g in range(NG)]
    q_g = [persist.tile([128, 2, G], FP8, name=f"q{g}", tag=f"q{g}")
           for g in range(NG)]
    rnb_g = [persist.tile([128, G], BF16, name=f"rnb{g}", tag=f"rnb{g}")
             for g in range(NG)]
    prow_g = [persist.tile([128, 16, D], BF16, name=f"pr{g}", tag=f"pr{g}")
              for g in range(NG)]
    ones8 = persist.tile([128, 2, 128], FP8, name="ones8", tag="ones8")
    rn_f = persist.tile([128, 64], F32, name="rn_f", tag="rn_f")
    sums = persist.tile([128, 24], F32, name="sums", tag="sums")
    rs = persist.tile([128, 8], F32, name="rs", tag="rs")
    esc_a = [persist.tile([128, 2, AW], FP8, name=f"esca{i}", tag=f"esca{i}")
             for i in range(2)]
    esc_b = [persist.tile([128, 2, BW], FP8, name=f"escb{i}", tag=f"escb{i}")
             for i in range(2)]
    dram_rn = dram.tile([B], BF16, name="dram_rn", tag="dram_rn")

    nc.vector.memset(ones8, 1.0)

    # ---- Prologue, per 2048-col group: load pt + row-major aux, col
    # norms^2 via x*x-with-accum (DVE 4x mode), Newton rsqrt, broadcast
    # 1/n, scale to the fp8 operand ----
    for g in range(NG):
        eng = nc.sync if g % 2 == 0 else nc.gpsimd
        eng.dma_start(out=pt_g[g], in_=pt_in[:, :, g * G:(g + 1) * G])
        nc.sync.dma_start(
            out=prow_g[g], in_=prow_in[:, g * 16:(g + 1) * 16, :]
        )
        trash = work.tile([128, D], BF16, name="trash", tag="trash", bufs=2)
        for u in range(16):
            nc.vector.scalar_tensor_tensor(
                out=trash, in0=prow_g[g][:, u, :], scalar=1.0,
                in1=prow_g[g][:, u, :], op0=ALU.mult, op1=ALU.mult,
                accum_out=rn_f[:, g * 16 + u:g * 16 + u + 1],
            )
        rn_g = work.tile([128, 16], F32, name="rn_g", tag="rn_g", bufs=2)
        _newton_rsqrt(nc, work, rn_g, rn_f[:, g * 16:(g + 1) * 16], iters=1)
        rnb16 = work.tile([128, 16], BF16, name="rnb16", tag="rnb16", bufs=2)
        nc.vector.tensor_copy(rnb16, rn_g)
        nc.sync.dma_start(
            out=dram_rn[g * G:(g + 1) * G].rearrange("(p w) -> p w", p=128),
            in_=rnb16,
        )
        nc.gpsimd.dma_start(
            out=rnb_g[g],
            in_=dram_rn[g * G:(g + 1) * G].partition_broadcast(128),
        )
        # fp8 operand: DVE takes [0,1152), Pool takes [1152,2048)
        for t in (0, 1):
            nc.vector.tensor_mul(
                q_g[g][:, t, 0:1152], pt_g[g][:, t, 0:1152],
                rnb_g[g][:, 0:1152],
            )
            nc.vector.tensor_mul(
                q_g[g][:, t, 1152:G], pt_g[g][:, t, 1152:G],
                rnb_g[g][:, 1152:G],
            )

    # ---- Main loop: fp8 DoubleRow matmuls + fused exp/row-sum ----
    # (side, lhsT col base, rhs window start, tile widths, esc pair tiles)
    sides = (
        (0, 0, (PTILE, PTILE, PTILE), esc_a),
        (4096, 4096, (PTILE, PTILE, 1024), esc_b),
    )
    slot = 0
    for row_off, win0, tiles_w, escp in sides:
        for m in range(4):
            lo = row_off + 128 * m
            lhsT = q_g[lo // G][:, :, lo % G:lo % G + 128]
            toff = 0
            for tw in tiles_w:
                ps = main_psum.tile([128, PTILE], F32, name="ps")
                for wi in range(tw // CHUNK):
                    col = win0 + toff + wi * CHUNK
                    cg, cin = col // G, col % G
                    nc.tensor.matmul(
                        ps[:, wi * CHUNK:(wi + 1) * CHUNK],
                        lhsT, q_g[cg][:, :, cin:cin + CHUNK],
                        start=True, stop=True, perf_mode=DR,
                    )
                nc.scalar.activation(
                    out=escp[m // 2][:, m % 2, toff:toff + tw],
                    in_=ps[:, 0:tw], func=AF.Exp, scale=2.0,
                    accum_out=sums[:, slot:slot + 1],
                )
                slot += 1
                toff += tw

    # ---- Column sums (transpose credit): DoubleRow ones-matmul over the
    # fp8 exp pairs, skipping each side's own diagonal block ----
    cs_sb = persist.tile([1, CS_A + CS_B], F32, name="cs_sb", tag="cs_sb")
    for escp, ncols, out_base in ((esc_a, CS_A, 0), (esc_b, CS_B, CS_A)):
        for wi in range(ncols // CHUNK):
            w = BLK + wi * CHUNK
            cs = cs_psum.tile([128, CHUNK], F32, name="cs")
            nc.tensor.matmul(cs, ones8, escp[0][:, :, w:w + CHUNK],
                             start=True, stop=False, perf_mode=DR)
            nc.tensor.matmul(cs, ones8, escp[1][:, :, w:w + CHUNK],
                             start=False, stop=True, perf_mode=DR)
            nc.vector.tensor_copy(
                cs_sb[:, out_base + wi * CHUNK:out_base + (wi + 1) * CHUNK],
                cs[0:1, :],
            )
    nc.sync.dma_start(
        out=cs_out.rearrange("(o w) -> o w", o=1), in_=cs_sb
    )

    # ---- Epilogue: per-(side,m) row sums over the 3 tile partials ----
    nc.vector.tensor_reduce(
        rs, sums.rearrange("p (x t) -> p x t", t=3),
        axis=mybir.AxisListType.X, op=ALU.add,
    )
    nc.sync.dma_start(out=rs_out, in_=rs)

    for p in (cs_psum, main_psum, dram, work, persist):
        p.release()


_BUILT = None


def _build():
    global _BUILT
    if _BUILT is None:
        nc = bacc.Bacc("TRN2", target_bir_lowering=False, debug=False,
                       num_devices=N_CORES)
        pt_in = nc.dram_tensor("pt_in", [128, 2, B], BF16,
                               kind="ExternalInput").ap()
        prow_in = nc.dram_tensor("prow_in", [128, 64, D], BF16,
                                 kind="ExternalInput").ap()
        rs_out = nc.dram_tensor("rs_out", [128, 8], F32,
                                kind="ExternalOutput").ap()
        cs_out = nc.dram_tensor("cs_out", [CS_A + CS_B], F32,
                                kind="ExternalOutput").ap()
        with tile.TileContext(nc) as tc:
            _emit(tc, pt_in, prow_in, rs_out, cs_out)
        nc.finalize()
        _BUILT = nc
    return _BUILT


def run_on_hw(P, **spmd_kwargs):
    import ml_dtypes

    nc = _build()
    pb = np.asarray(P).astype(ml_dtypes.bfloat16)           # [8192, 256] bf16
    ptb = np.ascontiguousarray(pb.T)                        # [256, 8192] bf16
    in_maps = []
    for c in range(N_CORES):
        ptl = np.roll(ptb, -BLK * c, axis=1)          # local col j = global 512c+j
        ptd = np.ascontiguousarray(
            ptl.reshape(2, 128, B).transpose(1, 0, 2)  # [128, 2, 8192], d=128t+p
        )
        # row-major aux for norms: prow[p, 16g+u, :] = P_local[2048g+16p+u, :]
        prl = np.roll(pb, -BLK * c, axis=0)
        prow = np.ascontiguousarray(
            prl.reshape(NG, 128, 16, D).transpose(1, 0, 2, 3).reshape(128, 64, D)
        )
        in_maps.append({"pt_in": ptd, "prow_in": prow})
    return bass_utils.run_bass_kernel_spmd(
        nc, in_maps, core_ids=list(range(N_CORES)), **spmd_kwargs
    )


def kernel(embedding1, embedding2, projection1, projection2):
    import jax.numpy as jnp

    # embeddings are unused by the reference computation
    P = np.ascontiguousarray(
        np.concatenate([projection1, projection2], axis=0), dtype=np.float32
    )
    res = run_on_hw(P)

    # Host assembly: add row partials and transpose (column-sum) partials.
    rowtot = np.zeros(B, np.float64)
    for c in range(N_CORES):
        base = BLK * c
        rsm = np.asarray(res.results[c]["rs_out"], np.float64)  # [128, 8]
        csm = np.asarray(res.results[c]["cs_out"], np.float64)  # [7680]
        for m in range(4):
            rowtot[base + 128 * m:base + 128 * (m + 1)] += rsm[:, m]
            b0 = (base + 4096 + 128 * m) % B
            rowtot[b0:b0 + 128] += rsm[:, 4 + m]
        # A-side col sums cover local cols [512, 4608)
        idx = (base + BLK + np.arange(CS_A)) % B
        np.add.at(rowtot, idx, csm[:CS_A])
        # B-side col sums cover local cols [4608, 8192)
        idx = (base + AW + np.arange(CS_B)) % B
        np.add.at(rowtot, idx, csm[CS_A:])

    # drop the self-similarity diagonal term exp(2*1)
    lse = np.log(rowtot - np.exp(2.0))
    # Reference fp32 semantics: logp_ii = f32(-2e9 - lse_i), then
    # loss = -mean(logp) with the platform's fp32 reduction.
    logp = (np.float32(-2.0e9) - lse.astype(np.float32)).astype(np.float32)
    loss = -jnp.mean(jnp.asarray(logp))
    return np.asarray(loss)
